# revision 3
# baseline (speedup 1.0000x reference)
"""TopK sparse autoencoder forward pass on 8 Trainium2 NeuronCores.

Math (reference):
    preact = (x - b_dec) @ W_enc.T + b_enc          # [B, F]
    top32 = exact per-row top-32 of relu(preact)
    x_hat = scatter(top32) @ W_dec.T + b_dec        # [B, D]

Strategy: data-parallel over batch rows (1024 rows/core, no collectives).
Per core:
  encode: fp32 matmul (PE), f-block-outer loop streaming W_encT once;
          per-block PSUM evacuated to a DRAM spill of preact; chunk-maxes
          (chunks of 64) reduced on DVE into cm[b-tile] [128, 256].
  topk:   exact hierarchical selection per 128-row tile:
          top-32 chunks by chunk-max (max/max_index/match_replace rounds)
          -> gather those 32*64 candidates from the DRAM spill
          -> 4 more rounds give the exact top-32 values + positions
          (top-32 elements provably live in the top-32 chunks).
  decode: compact sparse decode - gather the 32 selected rows of W_dec.T
          (fp16) per row via indirect DMA and accumulate
          sum_c diag(vals[:,c]) @ G_c on the PE (4x fewer FLOPs than a
          dense decode, no transpose of the encoded matrix needed).

Host side only reshapes/transposes inputs and converts W_dec.T to fp16.
"""
import sys
sys.path.insert(0, '/opt/trn_rl_repo')

import numpy as np

B, D, F, K = 8192, 768, 16384, 32
N_CORES = 8
BC = B // N_CORES          # rows per core (1024)
NBT = BC // 128            # b-tiles per core (8)
NFB = F // 512             # f-blocks (32)
KD = D // 128              # contraction chunks (6)
CH = 64                    # topk chunk width
C = F // CH                # chunks per row (256)
NCH = K // 8               # rounds of 8 (4)

ENC_MODE = "fp32"          # "fp32" (exact, 4cyc/row) or "f32r" (fast, lossy)

_cache = {}


def _fix_sync_waits(nc, maxw=1):
    """This container's walrus rejects >1 sync wait per instruction; split
    excess waits onto same-engine NoOps inserted just before."""
    import bass_rust
    import concourse.mybir as mybir
    ctr = 0
    for f in nc.m.functions:
        for bb in f.blocks:
            out, changed = [], False
            for inst in bb.instructions:
                si = inst.sync_info
                waits = list(si.on_wait) if si is not None else []
                if len(waits) > maxw:
                    changed = True
                    head, keep = waits[:-maxw], waits[-maxw:]
                    for i in range(0, len(head), maxw):
                        ctr += 1
                        nop = mybir.InstNoOp(
                            name=f"syncfix-nop-{id(nc)}-{ctr}", ins=[], outs=[])
                        nop.engine = inst.engine
                        nop.sync_info = bass_rust.SyncInfo(
                            on_wait=head[i:i + maxw], on_update=[])
                        out.append(nop)
                    si.on_wait = keep
                out.append(inst)
            if changed:
                bb.instructions = out


def _build(has_benc: bool, has_bdec: bool, repeat: int = 1):
    import concourse.bass as bass
    import concourse.mybir as mybir
    import concourse.tile as tile
    dt = mybir.dt
    Alu = mybir.AluOpType

    nc = bass.Bass("TRN2", target_bir_lowering=False, debug=False,
                   num_devices=N_CORES)

    xT_d = nc.dram_tensor("xT", [D, BC], dt.float32, kind="ExternalInput")
    wencT_d = nc.dram_tensor("wencT", [D, F], dt.float32, kind="ExternalInput")
    wdecT_d = nc.dram_tensor("wdecT16", [F, D], dt.float16, kind="ExternalInput")
    beff_d = nc.dram_tensor("beff", [1, F], dt.float32, kind="ExternalInput")
    bdec_d = nc.dram_tensor("bdec", [1, D], dt.float32, kind="ExternalInput")
    out_d = nc.dram_tensor("xhat", [BC, D], dt.float32, kind="ExternalOutput")
    preact_d = nc.dram_tensor("preact_spill", [BC, F], dt.float32)

    preact_flat = preact_d.ap().rearrange("b (c w) -> (b c) w", w=CH)

    def body(tc, ctx):
        sb = ctx.enter_context(tc.tile_pool(name="sb", bufs=2))
        sb1 = ctx.enter_context(tc.tile_pool(name="sb1", bufs=1))
        stp = ctx.enter_context(tc.tile_pool(name="stage", bufs=4))
        psA = ctx.enter_context(tc.tile_pool(name="psA", bufs=4, space="PSUM"))
        psB = ctx.enter_context(tc.tile_pool(name="psB", bufs=2, space="PSUM"))

        # resident inputs
        xT = sb1.tile([128, KD, BC], dt.float32)
        nc.sync.dma_start(xT[:], xT_d.ap().rearrange("(po pi) b -> pi po b", pi=128))
        if has_benc:
            beff = sb1.tile([1, F], dt.float32)
            nc.sync.dma_start(beff[:], beff_d.ap())
        if has_bdec:
            bdec1 = sb1.tile([1, D], dt.float32)
            nc.sync.dma_start(bdec1[:], bdec_d.ap())
            bdec_bc = sb1.tile([128, D], dt.float32)
            nc.gpsimd.partition_broadcast(bdec_bc[:], bdec1[:])

        cms = [sb1.tile([128, C], dt.float32, name=f"cm{bt}") for bt in range(NBT)]

        # ---------------- encode + spill + chunk-max ----------------
        wencT_v = wencT_d.ap().rearrange("(po pi) f -> pi po f", pi=128)
        for fb in range(NFB):
            wb = sb.tile([128, KD, 512], dt.float32, tag="wb")
            nc.sync.dma_start(wb[:], wencT_v[:, :, fb * 512:(fb + 1) * 512])
            if has_benc:
                beff_bc = stp.tile([128, 512], dt.float32, tag="beffbc")
                nc.gpsimd.partition_broadcast(
                    beff_bc[:], beff[:, fb * 512:(fb + 1) * 512])
            for bt in range(NBT):
                ps = psA.tile([128, 512], dt.float32, tag="encps")
                for k in range(KD):
                    lhsT = xT[:, k, bt * 128:(bt + 1) * 128]
                    rhs = wb[:, k, :]
                    if ENC_MODE == "f32r":
                        lhsT = lhsT.bitcast(dt.float32r)
                        rhs = rhs.bitcast(dt.float32r)
                    nc.tensor.matmul(ps[:], lhsT=lhsT, rhs=rhs,
                                     start=(k == 0), stop=(k == KD - 1))
                stage = stp.tile([128, 512], dt.float32, tag="stage")
                if has_benc:
                    nc.vector.tensor_add(stage[:], ps[:], beff_bc[:])
                    nc.vector.tensor_reduce(
                        cms[bt][:, fb * 8:(fb + 1) * 8],
                        stage[:].rearrange("p (c w) -> p c w", w=CH),
                        axis=mybir.AxisListType.X, op=Alu.max)
                else:
                    nc.scalar.copy(stage[:], ps[:])
                    nc.vector.tensor_reduce(
                        cms[bt][:, fb * 8:(fb + 1) * 8],
                        ps[:].rearrange("p (c w) -> p c w", w=CH),
                        axis=mybir.AxisListType.X, op=Alu.max)
                nc.sync.dma_start(
                    preact_d.ap()[bt * 128:(bt + 1) * 128,
                                  fb * 512:(fb + 1) * 512], stage[:])

        # iota constants for topk/index math
        jiota = sb1.tile([128, K], dt.uint32)
        nc.gpsimd.iota(jiota[:], pattern=[[1, K]], base=0, channel_multiplier=0)

        # ---------------- per-b-tile topk + compact decode ----------------
        for bt in range(NBT):
            # top-32 chunks by chunk max
            cmw = sb.tile([128, C], dt.float32, tag="cmw")
            nc.vector.tensor_copy(cmw[:], cms[bt][:])
            cm8 = sb.tile([128, 8], dt.float32, tag="cm8")
            chunkid = sb.tile([128, K], dt.uint32, tag="chunkid")
            for r in range(NCH):
                nc.vector.max(out=cm8[:], in_=cmw[:])
                nc.vector.max_index(out=chunkid[:, r * 8:(r + 1) * 8],
                                    in_max=cm8[:], in_values=cmw[:])
                if r != NCH - 1:
                    nc.vector.match_replace(out=cmw[:], in_to_replace=cm8[:],
                                            in_values=cmw[:], imm_value=-1e30)

            # gather the 32 chunks' contents from the DRAM spill
            rowoff = sb.tile([128, 1], dt.uint32, tag="rowoff")
            nc.gpsimd.iota(rowoff[:], pattern=[[1, 1]], base=bt * 128 * C,
                           channel_multiplier=C)
            off = sb.tile([128, K], dt.uint32, tag="off")
            nc.vector.tensor_tensor(off[:], chunkid[:],
                                    rowoff[:, :1].to_broadcast([128, K]),
                                    op=Alu.add)
            cand = sb.tile([128, K, CH], dt.float32, tag="cand")
            for j in range(K):
                nc.gpsimd.indirect_dma_start(
                    out=cand[:, j], out_offset=None,
                    in_=preact_flat,
                    in_offset=bass.IndirectOffsetOnAxis(ap=off[:, j:j + 1],
                                                        axis=0))

            # exact top-32 of the candidates
            candf = cand[:].rearrange("p a b -> p (a b)")
            vals = sb.tile([128, K], dt.float32, tag="vals")
            pos = sb.tile([128, K], dt.uint32, tag="pos")
            for r in range(NCH):
                m8 = vals[:, r * 8:(r + 1) * 8]
                nc.vector.max(out=m8, in_=candf)
                nc.vector.max_index(out=pos[:, r * 8:(r + 1) * 8],
                                    in_max=m8, in_values=candf)
                if r != NCH - 1:
                    nc.vector.match_replace(out=candf, in_to_replace=m8,
                                            in_values=candf, imm_value=-1e30)

            # positions -> global feature indices:
            # idx = chunkid[p, pos>>6]*64 + (pos&63), chunkid lookup done as a
            # one-hot compare-multiply-reduce (no per-partition gather on HW)
            j32 = sb.tile([128, K], dt.uint32, tag="j32")
            nc.vector.tensor_scalar(j32[:], pos[:], 6, None,
                                    op0=Alu.logical_shift_right)
            l32 = sb.tile([128, K], dt.uint32, tag="l32")
            nc.vector.tensor_scalar(l32[:], pos[:], CH - 1, None,
                                    op0=Alu.bitwise_and)
            eq = sb.tile([128, K, K], dt.uint32, tag="eq")
            nc.vector.tensor_tensor(
                eq[:], j32[:, :, None].to_broadcast([128, K, K]),
                jiota[:, None, :].to_broadcast([128, K, K]), op=Alu.is_equal)
            nc.vector.tensor_tensor(
                eq[:], eq[:], chunkid[:, None, :].to_broadcast([128, K, K]),
                op=Alu.mult)
            cs32 = sb.tile([128, K], dt.uint32, tag="cs32")
            nc.vector.tensor_reduce(cs32[:], eq[:],
                                    axis=mybir.AxisListType.X, op=Alu.max)
            idx32 = sb.tile([128, K], dt.uint32, tag="idx32")
            nc.vector.tensor_scalar(idx32[:], cs32[:], 6, None,
                                    op0=Alu.logical_shift_left)
            nc.vector.tensor_tensor(idx32[:], idx32[:], l32[:], op=Alu.add)

            # relu guard (if a row has <32 positive preacts, the reference's
            # extra top-k entries are relu zeros; zero coefficients match it)
            nc.vector.tensor_scalar_max(vals[:], vals[:], 0.0)

            # diag(vals[:, c]) tiles, fp16
            diag = sb.tile([128, K, 128], dt.float16, tag="diag")
            nc.gpsimd.affine_select(
                out=diag[:],
                in_=vals[:, :, None].to_broadcast([128, K, 128]),
                pattern=[[0, K], [1, 128]],
                compare_op=Alu.is_equal, fill=0.0,
                base=0, channel_multiplier=-1)

            # compact decode: xhat_tile = sum_c diag(vals[:,c]) @ WdecT[idx[:,c]]
            pso = psB.tile([128, D], dt.float32, tag="decps")
            for c in range(K):
                g = sb.tile([128, D], dt.float16, tag="g")
                nc.gpsimd.indirect_dma_start(
                    out=g[:], out_offset=None,
                    in_=wdecT_d.ap(),
                    in_offset=bass.IndirectOffsetOnAxis(ap=idx32[:, c:c + 1],
                                                        axis=0))
                nc.tensor.matmul(pso[:, :512], lhsT=diag[:, c, :],
                                 rhs=g[:, :512],
                                 start=(c == 0), stop=(c == K - 1))
                nc.tensor.matmul(pso[:, 512:D], lhsT=diag[:, c, :],
                                 rhs=g[:, 512:D],
                                 start=(c == 0), stop=(c == K - 1))
            osb = stp.tile([128, D], dt.float32, tag="osb")
            if has_bdec:
                nc.vector.tensor_add(osb[:], pso[:], bdec_bc[:])
            else:
                nc.scalar.copy(osb[:], pso[:])
            nc.sync.dma_start(out_d.ap()[bt * 128:(bt + 1) * 128, :], osb[:])

    from contextlib import ExitStack
    with tile.TileContext(nc) as tc:
        with ExitStack() as ctx:
            if repeat == 1:
                body(tc, ctx)
            else:
                with tc.For_i(0, repeat, 1):
                    body(tc, ctx)

    _fix_sync_waits(nc)
    return nc


def _get_runner(has_benc, has_bdec, repeat=1):
    key = (has_benc, has_bdec, repeat, ENC_MODE)
    if key in _cache:
        return _cache[key]
    import jax
    from jax.sharding import Mesh, PartitionSpec
    from jax.experimental.shard_map import shard_map
    import concourse.mybir as mybir
    from concourse import bass2jax
    from concourse.bass2jax import _bass_exec_p, install_neuronx_cc_hook

    nc = _build(has_benc, has_bdec, repeat)
    install_neuronx_cc_hook()

    partition_name = (nc.partition_id_tensor.name
                      if nc.partition_id_tensor else None)
    in_names, out_names, out_avals, zero_outs = [], [], [], []
    for alloc in nc.m.functions[0].allocations:
        if not isinstance(alloc, mybir.MemoryLocationSet):
            continue
        name = alloc.memorylocations[0].name
        if alloc.kind == "ExternalInput":
            if name != partition_name:
                in_names.append(name)
        elif alloc.kind == "ExternalOutput":
            shape = tuple(alloc.tensor_shape)
            dtype = mybir.dt.np(alloc.dtype)
            out_names.append(name)
            out_avals.append(jax.core.ShapedArray(shape, dtype))
            zero_outs.append(np.zeros(shape, dtype))
    n_params = len(in_names)
    all_in = in_names + out_names
    if partition_name is not None:
        all_in = all_in + [partition_name]

    def _bodyfn(*args):
        operands = list(args)
        if partition_name is not None:
            operands.append(bass2jax.partition_id_tensor())
        outs = _bass_exec_p.bind(
            *operands, out_avals=tuple(out_avals), in_names=tuple(all_in),
            out_names=tuple(out_names), lowering_input_output_aliases=(),
            sim_require_finite=True, sim_require_nnan=True, nc=nc)
        return tuple(outs)

    try:
        devices = jax.devices("axon")[:N_CORES]
    except Exception:
        devices = jax.devices()[:N_CORES]
    mesh = Mesh(np.asarray(devices), ("core",))
    n_outs = len(out_names)
    fn = jax.jit(
        shard_map(_bodyfn, mesh=mesh,
                  in_specs=(PartitionSpec("core"),) * (n_params + n_outs),
                  out_specs=(PartitionSpec("core"),) * n_outs,
                  check_rep=False),
        keep_unused=True)
    r = {"fn": fn, "in_names": in_names, "out_names": out_names,
         "zero_outs": zero_outs, "nc": nc}
    _cache[key] = r
    return r


def _prep_host(x, W_enc, b_enc, W_dec, b_dec):
    x_eff = x - b_dec[None, :]
    xT_full = np.ascontiguousarray(x_eff.T, dtype=np.float32)      # [D, B]
    wencT = np.ascontiguousarray(W_enc.T, dtype=np.float32)        # [D, F]
    wdecT16 = np.ascontiguousarray(W_dec.T, dtype=np.float16)      # [F, D]
    beff = (b_enc.astype(np.float64)
            - W_enc.astype(np.float64) @ b_dec.astype(np.float64))
    beff = beff.astype(np.float32)[None, :]                        # [1, F]
    bdec = b_dec.astype(np.float32)[None, :]                       # [1, D]
    return xT_full, wencT, wdecT16, beff, bdec


def kernel(x, W_enc, b_enc, W_dec, b_dec, _repeat=1, _timeit=False):
    x = np.asarray(x, np.float32)
    W_enc = np.asarray(W_enc, np.float32)
    b_enc = np.asarray(b_enc, np.float32)
    W_dec = np.asarray(W_dec, np.float32)
    b_dec = np.asarray(b_dec, np.float32)
    xT_full, wencT, wdecT16, beff, bdec = _prep_host(x, W_enc, b_enc, W_dec, b_dec)
    has_benc = bool(np.any(beff))
    has_bdec = bool(np.any(b_dec))
    r = _get_runner(has_benc, has_bdec, _repeat)

    per_core = {
        "xT": [np.ascontiguousarray(xT_full[:, c * BC:(c + 1) * BC])
               for c in range(N_CORES)],
        "wencT": [wencT] * N_CORES,
        "wdecT16": [wdecT16] * N_CORES,
        "beff": [beff] * N_CORES,
        "bdec": [bdec] * N_CORES,
    }
    args = [np.concatenate(per_core[name], axis=0) for name in r["in_names"]]
    args += [np.concatenate([z] * N_CORES, axis=0) for z in r["zero_outs"]]

    import jax, time
    dev_args = [jax.device_put(a) for a in args]
    outs = r["fn"](*dev_args)
    jax.block_until_ready(outs)
    if _timeit:
        times = []
        for _ in range(_timeit if isinstance(_timeit, int) and _timeit > 1 else 8):
            t0 = time.perf_counter()
            outs = r["fn"](*dev_args)
            jax.block_until_ready(outs)
            times.append(time.perf_counter() - t0)
        kernel.last_times = times

    xhat = np.asarray(outs[r["out_names"].index("xhat")])  # [B, D] concat
    return xhat.astype(np.float32)


# revision 15
# speedup vs baseline: 51.7924x; 51.7924x over previous
"""TopK sparse autoencoder forward pass on 8 Trainium2 NeuronCores.

Math (reference):
    preact = (x - b_dec) @ W_enc.T + b_enc          # [B, F]
    top32 = exact per-row top-32 of relu(preact)
    x_hat = scatter(top32) @ W_dec.T + b_dec        # [B, D]

Strategy: data-parallel over batch rows (1024 rows/core, no collectives).
Per core:
  encode: fp32 matmul (PE), f-block-outer loop streaming W_encT once;
          per-block PSUM evacuated to a DRAM spill of preact; chunk-maxes
          (chunks of 64) reduced on DVE into cm[b-tile] [128, 256].
  topk:   exact hierarchical selection per 128-row tile:
          top-32 chunks by chunk-max (max/max_index/match_replace rounds)
          -> gather those 32*64 candidates from the DRAM spill
          -> 4 more rounds give the exact top-32 values + positions
          (top-32 elements provably live in the top-32 chunks).
  decode: compact sparse decode - gather the 32 selected rows of W_dec.T
          (fp16) per row via indirect DMA and accumulate
          sum_c diag(vals[:,c]) @ G_c on the PE (4x fewer FLOPs than a
          dense decode, no transpose of the encoded matrix needed).

Host side only reshapes/transposes inputs and converts W_dec.T to fp16.
"""
import sys
sys.path.insert(0, '/opt/trn_rl_repo')

import numpy as np

B, D, F, K = 8192, 768, 16384, 32
N_CORES = 8
BC = B // N_CORES          # rows per core (1024)
NBT = BC // 128            # b-tiles per core (8)
NFB = F // 512             # f-blocks (32)
KD = D // 128              # contraction chunks (6)
CH = 64                    # topk chunk width
C = F // CH                # chunks per row (256)
NCH = K // 8               # rounds of 8 (4)

ENC_MODE = "bf16x3"          # "fp32" (exact, 4cyc/row) or "f32r" (fast, lossy)
HALVES = 1                 # split batch to overlap encode with topk/decode

_cache = {}


def _fix_sync_waits(nc, maxw=1):
    """This container's walrus rejects >1 sync wait per instruction; split
    excess waits onto same-engine NoOps inserted just before."""
    import bass_rust
    import concourse.mybir as mybir
    ctr = 0
    for f in nc.m.functions:
        for bb in f.blocks:
            out, changed = [], False
            for inst in bb.instructions:
                si = inst.sync_info
                waits = list(si.on_wait) if si is not None else []
                if len(waits) > maxw:
                    changed = True
                    head, keep = waits[:-maxw], waits[-maxw:]
                    for i in range(0, len(head), maxw):
                        ctr += 1
                        nop = mybir.InstNoOp(
                            name=f"syncfix-nop-{id(nc)}-{ctr}", ins=[], outs=[])
                        nop.engine = inst.engine
                        nop.sync_info = bass_rust.SyncInfo(
                            on_wait=head[i:i + maxw], on_update=[])
                        out.append(nop)
                    si.on_wait = keep
                out.append(inst)
            if changed:
                bb.instructions = out


def _build(has_benc: bool, has_bdec: bool, repeat: int = 1):
    import concourse.bass as bass
    import concourse.mybir as mybir
    import concourse.tile as tile
    dt = mybir.dt
    Alu = mybir.AluOpType

    nc = bass.Bass("TRN2", target_bir_lowering=False, debug=False,
                   num_devices=N_CORES)

    if ENC_MODE == "bf16x3":
        xTh_d = nc.dram_tensor("xTh", [D, BC], dt.bfloat16, kind="ExternalInput")
        xTl_d = nc.dram_tensor("xTl", [D, BC], dt.bfloat16, kind="ExternalInput")
        wencTh_d = nc.dram_tensor("wencTh", [D, F], dt.bfloat16,
                                  kind="ExternalInput")
        wencTl_d = nc.dram_tensor("wencTl", [D, F], dt.bfloat16,
                                  kind="ExternalInput")
    else:
        xT_d = nc.dram_tensor("xT", [D, BC], dt.float32, kind="ExternalInput")
        wencT_d = nc.dram_tensor("wencT", [D, F], dt.float32,
                                 kind="ExternalInput")
    wdecT_d = nc.dram_tensor("wdecT16", [F, D], dt.float16, kind="ExternalInput")
    beff_d = nc.dram_tensor("beff", [1, F], dt.float32, kind="ExternalInput")
    bdec_d = nc.dram_tensor("bdec", [1, D], dt.float32, kind="ExternalInput")
    out_d = nc.dram_tensor("xhat", [BC, D], dt.float32, kind="ExternalOutput")
    preact_d = nc.dram_tensor("preact_spill", [BC, F], dt.float32)

    preact_flat = preact_d.ap().rearrange("b (c w) -> (b c) w", w=CH)

    def body(tc, pools):
        sb, sb1, stp, psA, psB = pools

        # resident inputs
        if ENC_MODE == "bf16x3":
            xTh = sb1.tile([128, KD, BC], dt.bfloat16)
            nc.sync.dma_start(
                xTh[:], xTh_d.ap().rearrange("(po pi) b -> pi po b", pi=128))
            xTl = sb1.tile([128, KD, BC], dt.bfloat16)
            nc.sync.dma_start(
                xTl[:], xTl_d.ap().rearrange("(po pi) b -> pi po b", pi=128))
        else:
            xT = sb1.tile([128, KD, BC], dt.float32)
            nc.sync.dma_start(
                xT[:], xT_d.ap().rearrange("(po pi) b -> pi po b", pi=128))
        if has_benc:
            beff = sb1.tile([1, F], dt.float32)
            nc.sync.dma_start(beff[:], beff_d.ap())
        if has_bdec:
            bdec1 = sb1.tile([1, D], dt.float32)
            nc.sync.dma_start(bdec1[:], bdec_d.ap())
            bdec_bc = sb1.tile([128, D], dt.float32)
            nc.gpsimd.partition_broadcast(bdec_bc[:], bdec1[:])

        cms = [sb1.tile([128, C], dt.float32, name=f"cm{bt}") for bt in range(NBT)]

        # ---------------- encode + spill + chunk-max ----------------
        if ENC_MODE == "bf16x3":
            wh_v = wencTh_d.ap().rearrange("(po pi) f -> pi po f", pi=128)
            wl_v = wencTl_d.ap().rearrange("(po pi) f -> pi po f", pi=128)
        else:
            wencT_v = wencT_d.ap().rearrange("(po pi) f -> pi po f", pi=128)
        def encode_blocks(bts):
          for fb in range(NFB):
            if ENC_MODE == "bf16x3":
                wbh = sb.tile([128, KD, 512], dt.bfloat16, tag="wbh")
                nc.sync.dma_start(wbh[:], wh_v[:, :, fb * 512:(fb + 1) * 512])
                wbl = sb.tile([128, KD, 512], dt.bfloat16, tag="wbl")
                nc.sync.dma_start(wbl[:], wl_v[:, :, fb * 512:(fb + 1) * 512])
            else:
                wb = sb.tile([128, KD, 512], dt.float32, tag="wb")
                nc.sync.dma_start(wb[:], wencT_v[:, :, fb * 512:(fb + 1) * 512])
            if has_benc:
                beff_bc = stp.tile([128, 512], dt.float32, tag="beffbc")
                nc.gpsimd.partition_broadcast(
                    beff_bc[:], beff[:, fb * 512:(fb + 1) * 512])
            for bt in bts:
                ps = psA.tile([128, 512], dt.float32, tag="encps")
                if ENC_MODE == "bf16x3":
                    bsl = slice(bt * 128, (bt + 1) * 128)
                    terms = [(xTh, wbh), (xTh, wbl), (xTl, wbh)]
                    n_mm = KD * len(terms)
                    i = 0
                    for k in range(KD):
                        for (a, w_) in terms:
                            nc.tensor.matmul(ps[:], lhsT=a[:, k, bsl],
                                             rhs=w_[:, k, :],
                                             start=(i == 0), stop=(i == n_mm - 1))
                            i += 1
                else:
                    for k in range(KD):
                        lhsT = xT[:, k, bt * 128:(bt + 1) * 128]
                        rhs = wb[:, k, :]
                        if ENC_MODE == "f32r":
                            lhsT = lhsT.bitcast(dt.float32r)
                            rhs = rhs.bitcast(dt.float32r)
                        nc.tensor.matmul(ps[:], lhsT=lhsT, rhs=rhs,
                                         start=(k == 0), stop=(k == KD - 1))
                stage = stp.tile([128, 512], dt.float32, tag="stage")
                if has_benc:
                    nc.vector.tensor_add(stage[:], ps[:], beff_bc[:])
                    nc.vector.tensor_reduce(
                        cms[bt][:, fb * 8:(fb + 1) * 8],
                        stage[:].rearrange("p (c w) -> p c w", w=CH),
                        axis=mybir.AxisListType.X, op=Alu.max)
                else:
                    nc.scalar.copy(stage[:], ps[:])
                    nc.vector.tensor_reduce(
                        cms[bt][:, fb * 8:(fb + 1) * 8],
                        ps[:].rearrange("p (c w) -> p c w", w=CH),
                        axis=mybir.AxisListType.X, op=Alu.max)
                nc.sync.dma_start(
                    preact_d.ap()[bt * 128:(bt + 1) * 128,
                                  fb * 512:(fb + 1) * 512], stage[:])

        # iota constants for topk/index math
        jiota = sb1.tile([128, K], dt.uint32)
        nc.gpsimd.iota(jiota[:], pattern=[[1, K]], base=0, channel_multiplier=0)

        # ---------------- per-b-tile topk + compact decode ----------------
        def tail(bt):
            # top-32 chunks by chunk max
            cmw = sb.tile([128, C], dt.float32, tag="cmw")
            nc.vector.tensor_copy(cmw[:], cms[bt][:])
            cm8 = sb.tile([128, 8], dt.float32, tag="cm8")
            chunkid = sb.tile([128, K], dt.uint32, tag="chunkid")
            for r in range(NCH):
                nc.vector.max(out=cm8[:], in_=cmw[:])
                nc.vector.max_index(out=chunkid[:, r * 8:(r + 1) * 8],
                                    in_max=cm8[:], in_values=cmw[:])
                if r != NCH - 1:
                    nc.vector.match_replace(out=cmw[:], in_to_replace=cm8[:],
                                            in_values=cmw[:], imm_value=-1e30)

            # gather the 32 chunks' contents from the DRAM spill
            rowoff = sb.tile([128, 1], dt.uint32, tag="rowoff")
            nc.gpsimd.iota(rowoff[:], pattern=[[1, 1]], base=bt * 128 * C,
                           channel_multiplier=C)
            off = sb.tile([128, K], dt.uint32, tag="off")
            nc.vector.tensor_tensor(off[:], chunkid[:],
                                    rowoff[:, :1].to_broadcast([128, K]),
                                    op=Alu.add)
            cand = sb.tile([128, K, CH], dt.float32, tag="cand")
            for j in range(K):
                nc.gpsimd.indirect_dma_start(
                    out=cand[:, j], out_offset=None,
                    in_=preact_flat,
                    in_offset=bass.IndirectOffsetOnAxis(ap=off[:, j:j + 1],
                                                        axis=0))

            # exact top-32 of the candidates
            candf = cand[:].rearrange("p a b -> p (a b)")
            vals = sb.tile([128, K], dt.float32, tag="vals")
            pos = sb.tile([128, K], dt.uint32, tag="pos")
            for r in range(NCH):
                m8 = vals[:, r * 8:(r + 1) * 8]
                nc.vector.max(out=m8, in_=candf)
                nc.vector.max_index(out=pos[:, r * 8:(r + 1) * 8],
                                    in_max=m8, in_values=candf)
                if r != NCH - 1:
                    nc.vector.match_replace(out=candf, in_to_replace=m8,
                                            in_values=candf, imm_value=-1e30)

            # positions -> global feature indices:
            # idx = chunkid[p, pos>>6]*64 + (pos&63), chunkid lookup done as a
            # one-hot compare-multiply-reduce (no per-partition gather on HW)
            j32 = sb.tile([128, K], dt.uint32, tag="j32")
            nc.vector.tensor_scalar(j32[:], pos[:], 6, None,
                                    op0=Alu.logical_shift_right)
            l32 = sb.tile([128, K], dt.uint32, tag="l32")
            nc.vector.tensor_scalar(l32[:], pos[:], CH - 1, None,
                                    op0=Alu.bitwise_and)
            eq = sb.tile([128, K, K], dt.uint32, tag="eq")
            nc.vector.tensor_tensor(
                eq[:], j32[:, :, None].to_broadcast([128, K, K]),
                jiota[:, None, :].to_broadcast([128, K, K]), op=Alu.is_equal)
            nc.vector.tensor_tensor(
                eq[:], eq[:], chunkid[:, None, :].to_broadcast([128, K, K]),
                op=Alu.mult)
            cs32 = sb.tile([128, K], dt.uint32, tag="cs32")
            nc.vector.tensor_reduce(cs32[:], eq[:],
                                    axis=mybir.AxisListType.X, op=Alu.max)
            idx32 = sb.tile([128, K], dt.uint32, tag="idx32")
            nc.vector.tensor_scalar(idx32[:], cs32[:], 6, None,
                                    op0=Alu.logical_shift_left)
            nc.vector.tensor_tensor(idx32[:], idx32[:], l32[:], op=Alu.add)

            # relu guard (if a row has <32 positive preacts, the reference's
            # extra top-k entries are relu zeros; zero coefficients match it)
            nc.vector.tensor_scalar_max(vals[:], vals[:], 0.0)

            # diag(vals[:, c]) tiles, fp16
            diag = sb.tile([128, K, 128], dt.float16, tag="diag")
            nc.gpsimd.affine_select(
                out=diag[:],
                in_=vals[:, :, None].to_broadcast([128, K, 128]),
                pattern=[[0, K], [1, 128]],
                compare_op=Alu.is_equal, fill=0.0,
                base=0, channel_multiplier=-1)

            # compact decode: xhat_tile = sum_c diag(vals[:,c]) @ WdecT[idx[:,c]]
            pso = psB.tile([128, D], dt.float32, tag="decps")
            for c in range(K):
                g = sb.tile([128, D], dt.float16, tag="g")
                nc.gpsimd.indirect_dma_start(
                    out=g[:], out_offset=None,
                    in_=wdecT_d.ap(),
                    in_offset=bass.IndirectOffsetOnAxis(ap=idx32[:, c:c + 1],
                                                        axis=0))
                nc.tensor.matmul(pso[:, :512], lhsT=diag[:, c, :],
                                 rhs=g[:, :512],
                                 start=(c == 0), stop=(c == K - 1))
                nc.tensor.matmul(pso[:, 512:D], lhsT=diag[:, c, :],
                                 rhs=g[:, 512:D],
                                 start=(c == 0), stop=(c == K - 1))
            osb = stp.tile([128, D], dt.float32, tag="osb")
            if has_bdec:
                nc.vector.tensor_add(osb[:], pso[:], bdec_bc[:])
            else:
                nc.scalar.copy(osb[:], pso[:])
            nc.sync.dma_start(out_d.ap()[bt * 128:(bt + 1) * 128, :], osb[:])

        if HALVES == 1:
            groups = [list(range(NBT))]
        else:
            h = NBT // HALVES
            groups = [list(range(i * h, (i + 1) * h)) for i in range(HALVES)]
        for bts in groups:
            encode_blocks(bts)
            for bt in bts:
                tail(bt)

    from contextlib import ExitStack
    with tile.TileContext(nc) as tc:
        with ExitStack() as ctx:
            pools = (
                ctx.enter_context(tc.tile_pool(name="sb", bufs=2)),
                ctx.enter_context(tc.tile_pool(name="sb1", bufs=1)),
                ctx.enter_context(tc.tile_pool(name="stage", bufs=4)),
                ctx.enter_context(tc.tile_pool(name="psA", bufs=4, space="PSUM")),
                ctx.enter_context(tc.tile_pool(name="psB", bufs=2, space="PSUM")),
            )
            if repeat == 1:
                body(tc, pools)
            else:
                with tc.For_i(0, repeat, 1):
                    body(tc, pools)

    _fix_sync_waits(nc)
    return nc


def _get_runner(has_benc, has_bdec, repeat=1):
    key = (has_benc, has_bdec, repeat, ENC_MODE, HALVES)
    if key in _cache:
        return _cache[key]
    import jax
    from jax.sharding import Mesh, PartitionSpec
    from jax.experimental.shard_map import shard_map
    import concourse.mybir as mybir
    from concourse import bass2jax
    from concourse.bass2jax import _bass_exec_p, install_neuronx_cc_hook

    nc = _build(has_benc, has_bdec, repeat)
    install_neuronx_cc_hook()

    partition_name = (nc.partition_id_tensor.name
                      if nc.partition_id_tensor else None)
    in_names, out_names, out_avals, zero_outs = [], [], [], []
    for alloc in nc.m.functions[0].allocations:
        if not isinstance(alloc, mybir.MemoryLocationSet):
            continue
        name = alloc.memorylocations[0].name
        if alloc.kind == "ExternalInput":
            if name != partition_name:
                in_names.append(name)
        elif alloc.kind == "ExternalOutput":
            shape = tuple(alloc.tensor_shape)
            dtype = mybir.dt.np(alloc.dtype)
            out_names.append(name)
            out_avals.append(jax.core.ShapedArray(shape, dtype))
            zero_outs.append(np.zeros(shape, dtype))
    n_params = len(in_names)
    all_in = in_names + out_names
    if partition_name is not None:
        all_in = all_in + [partition_name]

    def _bodyfn(*args):
        operands = list(args)
        if partition_name is not None:
            operands.append(bass2jax.partition_id_tensor())
        outs = _bass_exec_p.bind(
            *operands, out_avals=tuple(out_avals), in_names=tuple(all_in),
            out_names=tuple(out_names), lowering_input_output_aliases=(),
            sim_require_finite=True, sim_require_nnan=True, nc=nc)
        return tuple(outs)

    try:
        devices = jax.devices("axon")[:N_CORES]
    except Exception:
        devices = jax.devices()[:N_CORES]
    mesh = Mesh(np.asarray(devices), ("core",))
    n_outs = len(out_names)
    fn = jax.jit(
        shard_map(_bodyfn, mesh=mesh,
                  in_specs=(PartitionSpec("core"),) * (n_params + n_outs),
                  out_specs=(PartitionSpec("core"),) * n_outs,
                  check_rep=False),
        keep_unused=True)
    sharding = jax.sharding.NamedSharding(mesh, PartitionSpec("core"))
    r = {"fn": fn, "in_names": in_names, "out_names": out_names,
         "zero_outs": zero_outs, "nc": nc, "sharding": sharding}
    _cache[key] = r
    return r


def _prep_host(x, W_enc, b_enc, W_dec, b_dec):
    x_eff = x - b_dec[None, :]
    xT_full = np.ascontiguousarray(x_eff.T, dtype=np.float32)      # [D, B]
    wencT = np.ascontiguousarray(W_enc.T, dtype=np.float32)        # [D, F]
    wdecT16 = np.ascontiguousarray(W_dec.T, dtype=np.float16)      # [F, D]
    beff = (b_enc.astype(np.float64)
            - W_enc.astype(np.float64) @ b_dec.astype(np.float64))
    beff = beff.astype(np.float32)[None, :]                        # [1, F]
    bdec = b_dec.astype(np.float32)[None, :]                       # [1, D]
    return xT_full, wencT, wdecT16, beff, bdec


def kernel(x, W_enc, b_enc, W_dec, b_dec, _repeat=1, _timeit=False):
    x = np.asarray(x, np.float32)
    W_enc = np.asarray(W_enc, np.float32)
    b_enc = np.asarray(b_enc, np.float32)
    W_dec = np.asarray(W_dec, np.float32)
    b_dec = np.asarray(b_dec, np.float32)
    xT_full, wencT, wdecT16, beff, bdec = _prep_host(x, W_enc, b_enc, W_dec, b_dec)
    has_benc = bool(np.any(beff))
    has_bdec = bool(np.any(b_dec))
    r = _get_runner(has_benc, has_bdec, _repeat)

    per_core = {
        "wdecT16": [wdecT16] * N_CORES,
        "beff": [beff] * N_CORES,
        "bdec": [bdec] * N_CORES,
    }
    if ENC_MODE == "bf16x3":
        import ml_dtypes
        bf16 = ml_dtypes.bfloat16
        xTh = xT_full.astype(bf16)
        xTl = (xT_full - xTh.astype(np.float32)).astype(bf16)
        wh = wencT.astype(bf16)
        wl = (wencT - wh.astype(np.float32)).astype(bf16)
        per_core["xTh"] = [np.ascontiguousarray(xTh[:, c * BC:(c + 1) * BC])
                           for c in range(N_CORES)]
        per_core["xTl"] = [np.ascontiguousarray(xTl[:, c * BC:(c + 1) * BC])
                           for c in range(N_CORES)]
        per_core["wencTh"] = [wh] * N_CORES
        per_core["wencTl"] = [wl] * N_CORES
    else:
        per_core["xT"] = [np.ascontiguousarray(xT_full[:, c * BC:(c + 1) * BC])
                          for c in range(N_CORES)]
        per_core["wencT"] = [wencT] * N_CORES
    args = [np.concatenate(per_core[name], axis=0) for name in r["in_names"]]
    args += [np.concatenate([z] * N_CORES, axis=0) for z in r["zero_outs"]]

    import jax, time
    dev_args = [jax.device_put(a, r["sharding"]) for a in args]
    kernel.last_dev_args = dev_args
    kernel.last_runner = r
    outs = r["fn"](*dev_args)
    jax.block_until_ready(outs)
    if _timeit:
        times = []
        for _ in range(_timeit if isinstance(_timeit, int) and _timeit > 1 else 8):
            t0 = time.perf_counter()
            outs = r["fn"](*dev_args)
            jax.block_until_ready(outs)
            times.append(time.perf_counter() - t0)
        kernel.last_times = times

    xhat = np.asarray(outs[r["out_names"].index("xhat")])  # [B, D] concat
    return xhat.astype(np.float32)


# revision 16
# speedup vs baseline: 57.4730x; 1.1097x over previous
"""TopK sparse autoencoder forward pass on 8 Trainium2 NeuronCores.

Math (reference):
    preact = (x - b_dec) @ W_enc.T + b_enc          # [B, F]
    top32 = exact per-row top-32 of relu(preact)
    x_hat = scatter(top32) @ W_dec.T + b_dec        # [B, D]

Strategy: data-parallel over batch rows (1024 rows/core, no collectives).
Per core:
  encode: fp32 matmul (PE), f-block-outer loop streaming W_encT once;
          per-block PSUM evacuated to a DRAM spill of preact; chunk-maxes
          (chunks of 64) reduced on DVE into cm[b-tile] [128, 256].
  topk:   exact hierarchical selection per 128-row tile:
          top-32 chunks by chunk-max (max/max_index/match_replace rounds)
          -> gather those 32*64 candidates from the DRAM spill
          -> 4 more rounds give the exact top-32 values + positions
          (top-32 elements provably live in the top-32 chunks).
  decode: compact sparse decode - gather the 32 selected rows of W_dec.T
          (fp16) per row via indirect DMA and accumulate
          sum_c diag(vals[:,c]) @ G_c on the PE (4x fewer FLOPs than a
          dense decode, no transpose of the encoded matrix needed).

Host side only reshapes/transposes inputs and converts W_dec.T to fp16.
"""
import sys
sys.path.insert(0, '/opt/trn_rl_repo')

import numpy as np

B, D, F, K = 8192, 768, 16384, 32
N_CORES = 8
BC = B // N_CORES          # rows per core (1024)
NBT = BC // 128            # b-tiles per core (8)
NFB = F // 512             # f-blocks (32)
KD = D // 128              # contraction chunks (6)
CH = 64                    # topk chunk width
C = F // CH                # chunks per row (256)
NCH = K // 8               # rounds of 8 (4)

ENC_MODE = "fp32"          # "fp32" (exact, 4cyc/row) or "f32r" (fast, lossy)
HALVES = 1                 # split batch to overlap encode with topk/decode

_cache = {}


def _fix_sync_waits(nc, maxw=1):
    """This container's walrus rejects >1 sync wait per instruction; split
    excess waits onto same-engine NoOps inserted just before."""
    import bass_rust
    import concourse.mybir as mybir
    ctr = 0
    for f in nc.m.functions:
        for bb in f.blocks:
            out, changed = [], False
            for inst in bb.instructions:
                si = inst.sync_info
                waits = list(si.on_wait) if si is not None else []
                if len(waits) > maxw:
                    changed = True
                    head, keep = waits[:-maxw], waits[-maxw:]
                    for i in range(0, len(head), maxw):
                        ctr += 1
                        nop = mybir.InstNoOp(
                            name=f"syncfix-nop-{id(nc)}-{ctr}", ins=[], outs=[])
                        nop.engine = inst.engine
                        nop.sync_info = bass_rust.SyncInfo(
                            on_wait=head[i:i + maxw], on_update=[])
                        out.append(nop)
                    si.on_wait = keep
                out.append(inst)
            if changed:
                bb.instructions = out


def _build(has_benc: bool, has_bdec: bool, repeat: int = 1):
    import concourse.bass as bass
    import concourse.mybir as mybir
    import concourse.tile as tile
    dt = mybir.dt
    Alu = mybir.AluOpType

    nc = bass.Bass("TRN2", target_bir_lowering=False, debug=False,
                   num_devices=N_CORES)

    if ENC_MODE == "bf16x3":
        xTh_d = nc.dram_tensor("xTh", [D, BC], dt.bfloat16, kind="ExternalInput")
        xTl_d = nc.dram_tensor("xTl", [D, BC], dt.bfloat16, kind="ExternalInput")
        wencTh_d = nc.dram_tensor("wencTh", [D, F], dt.bfloat16,
                                  kind="ExternalInput")
        wencTl_d = nc.dram_tensor("wencTl", [D, F], dt.bfloat16,
                                  kind="ExternalInput")
    else:
        xT_d = nc.dram_tensor("xT", [D, BC], dt.float32, kind="ExternalInput")
        wencT_d = nc.dram_tensor("wencT", [D, F], dt.float32,
                                 kind="ExternalInput")
    wdecT_d = nc.dram_tensor("wdecT16", [F, D], dt.float16, kind="ExternalInput")
    beff_d = nc.dram_tensor("beff", [1, F], dt.float32, kind="ExternalInput")
    bdec_d = nc.dram_tensor("bdec", [1, D], dt.float32, kind="ExternalInput")
    out_d = nc.dram_tensor("xhat", [BC, D], dt.float32, kind="ExternalOutput")
    preact_d = nc.dram_tensor("preact_spill", [BC, F], dt.float32)

    preact_flat = preact_d.ap().rearrange("b (c w) -> (b c) w", w=CH)

    def body(tc, pools):
        sb, sb1, stp, psA, psB = pools

        # resident inputs
        if ENC_MODE == "bf16x3":
            xTh = sb1.tile([128, KD, BC], dt.bfloat16)
            nc.sync.dma_start(
                xTh[:], xTh_d.ap().rearrange("(po pi) b -> pi po b", pi=128))
            xTl = sb1.tile([128, KD, BC], dt.bfloat16)
            nc.sync.dma_start(
                xTl[:], xTl_d.ap().rearrange("(po pi) b -> pi po b", pi=128))
        else:
            xT = sb1.tile([128, KD, BC], dt.float32)
            nc.sync.dma_start(
                xT[:], xT_d.ap().rearrange("(po pi) b -> pi po b", pi=128))
        if has_benc:
            beff = sb1.tile([1, F], dt.float32)
            nc.sync.dma_start(beff[:], beff_d.ap())
        if has_bdec:
            bdec1 = sb1.tile([1, D], dt.float32)
            nc.sync.dma_start(bdec1[:], bdec_d.ap())
            bdec_bc = sb1.tile([128, D], dt.float32)
            nc.gpsimd.partition_broadcast(bdec_bc[:], bdec1[:])

        cms = [sb1.tile([128, C], dt.float32, name=f"cm{bt}") for bt in range(NBT)]

        # ---------------- encode + spill + chunk-max ----------------
        if ENC_MODE == "bf16x3":
            wh_v = wencTh_d.ap().rearrange("(po pi) f -> pi po f", pi=128)
            wl_v = wencTl_d.ap().rearrange("(po pi) f -> pi po f", pi=128)
        else:
            wencT_v = wencT_d.ap().rearrange("(po pi) f -> pi po f", pi=128)
        def encode_blocks(bts):
          for fb in range(NFB):
            if ENC_MODE == "bf16x3":
                wbh = sb.tile([128, KD, 512], dt.bfloat16, tag="wbh")
                nc.sync.dma_start(wbh[:], wh_v[:, :, fb * 512:(fb + 1) * 512])
                wbl = sb.tile([128, KD, 512], dt.bfloat16, tag="wbl")
                nc.sync.dma_start(wbl[:], wl_v[:, :, fb * 512:(fb + 1) * 512])
            else:
                wb = sb.tile([128, KD, 512], dt.float32, tag="wb")
                nc.sync.dma_start(wb[:], wencT_v[:, :, fb * 512:(fb + 1) * 512])
            if has_benc:
                beff_bc = stp.tile([128, 512], dt.float32, tag="beffbc")
                nc.gpsimd.partition_broadcast(
                    beff_bc[:], beff[:, fb * 512:(fb + 1) * 512])
            for bt in bts:
                ps = psA.tile([128, 512], dt.float32, tag="encps")
                if ENC_MODE == "bf16x3":
                    bsl = slice(bt * 128, (bt + 1) * 128)
                    terms = [(xTh, wbh), (xTh, wbl), (xTl, wbh)]
                    n_mm = KD * len(terms)
                    i = 0
                    for k in range(KD):
                        for (a, w_) in terms:
                            nc.tensor.matmul(ps[:], lhsT=a[:, k, bsl],
                                             rhs=w_[:, k, :],
                                             start=(i == 0), stop=(i == n_mm - 1))
                            i += 1
                else:
                    for k in range(KD):
                        lhsT = xT[:, k, bt * 128:(bt + 1) * 128]
                        rhs = wb[:, k, :]
                        if ENC_MODE == "f32r":
                            lhsT = lhsT.bitcast(dt.float32r)
                            rhs = rhs.bitcast(dt.float32r)
                        nc.tensor.matmul(ps[:], lhsT=lhsT, rhs=rhs,
                                         start=(k == 0), stop=(k == KD - 1))
                stage = stp.tile([128, 512], dt.float32, tag="stage")
                if has_benc:
                    nc.vector.tensor_add(stage[:], ps[:], beff_bc[:])
                    nc.vector.tensor_reduce(
                        cms[bt][:, fb * 8:(fb + 1) * 8],
                        stage[:].rearrange("p (c w) -> p c w", w=CH),
                        axis=mybir.AxisListType.X, op=Alu.max)
                else:
                    nc.scalar.copy(stage[:], ps[:])
                    nc.vector.tensor_reduce(
                        cms[bt][:, fb * 8:(fb + 1) * 8],
                        ps[:].rearrange("p (c w) -> p c w", w=CH),
                        axis=mybir.AxisListType.X, op=Alu.max)
                nc.sync.dma_start(
                    preact_d.ap()[bt * 128:(bt + 1) * 128,
                                  fb * 512:(fb + 1) * 512], stage[:])

        # iota constants for topk/index math
        jiota = sb1.tile([128, K], dt.uint32)
        nc.gpsimd.iota(jiota[:], pattern=[[1, K]], base=0, channel_multiplier=0)

        # ---------------- per-b-tile topk + compact decode ----------------
        def tail(bt):
            # top-32 chunks by chunk max
            cmw = sb.tile([128, C], dt.float32, tag="cmw")
            nc.vector.tensor_copy(cmw[:], cms[bt][:])
            cm8 = sb.tile([128, 8], dt.float32, tag="cm8")
            chunkid = sb.tile([128, K], dt.uint32, tag="chunkid")
            for r in range(NCH):
                nc.vector.max(out=cm8[:], in_=cmw[:])
                nc.vector.max_index(out=chunkid[:, r * 8:(r + 1) * 8],
                                    in_max=cm8[:], in_values=cmw[:])
                if r != NCH - 1:
                    nc.vector.match_replace(out=cmw[:], in_to_replace=cm8[:],
                                            in_values=cmw[:], imm_value=-1e30)

            # gather the 32 chunks' contents from the DRAM spill
            rowoff = sb.tile([128, 1], dt.uint32, tag="rowoff")
            nc.gpsimd.iota(rowoff[:], pattern=[[1, 1]], base=bt * 128 * C,
                           channel_multiplier=C)
            off = sb.tile([128, K], dt.uint32, tag="off")
            nc.vector.tensor_tensor(off[:], chunkid[:],
                                    rowoff[:, :1].to_broadcast([128, K]),
                                    op=Alu.add)
            cand = sb.tile([128, K, CH], dt.float32, tag="cand")
            for j in range(K):
                nc.gpsimd.indirect_dma_start(
                    out=cand[:, j], out_offset=None,
                    in_=preact_flat,
                    in_offset=bass.IndirectOffsetOnAxis(ap=off[:, j:j + 1],
                                                        axis=0))

            # exact top-32 of the candidates
            candf = cand[:].rearrange("p a b -> p (a b)")
            vals = sb.tile([128, K], dt.float32, tag="vals")
            pos = sb.tile([128, K], dt.uint32, tag="pos")
            for r in range(NCH):
                m8 = vals[:, r * 8:(r + 1) * 8]
                nc.vector.max(out=m8, in_=candf)
                nc.vector.max_index(out=pos[:, r * 8:(r + 1) * 8],
                                    in_max=m8, in_values=candf)
                if r != NCH - 1:
                    nc.vector.match_replace(out=candf, in_to_replace=m8,
                                            in_values=candf, imm_value=-1e30)

            # positions -> global feature indices:
            # idx = chunkid[p, pos>>6]*64 + (pos&63), chunkid lookup done as a
            # one-hot compare-multiply-reduce (no per-partition gather on HW)
            j32 = sb.tile([128, K], dt.uint32, tag="j32")
            nc.vector.tensor_scalar(j32[:], pos[:], 6, None,
                                    op0=Alu.logical_shift_right)
            l32 = sb.tile([128, K], dt.uint32, tag="l32")
            nc.vector.tensor_scalar(l32[:], pos[:], CH - 1, None,
                                    op0=Alu.bitwise_and)
            eq = sb.tile([128, K, K], dt.uint32, tag="eq")
            nc.vector.tensor_tensor(
                eq[:], j32[:, :, None].to_broadcast([128, K, K]),
                jiota[:, None, :].to_broadcast([128, K, K]), op=Alu.is_equal)
            nc.vector.tensor_tensor(
                eq[:], eq[:], chunkid[:, None, :].to_broadcast([128, K, K]),
                op=Alu.mult)
            cs32 = sb.tile([128, K], dt.uint32, tag="cs32")
            nc.vector.tensor_reduce(cs32[:], eq[:],
                                    axis=mybir.AxisListType.X, op=Alu.max)
            idx32 = sb.tile([128, K], dt.uint32, tag="idx32")
            nc.vector.tensor_scalar(idx32[:], cs32[:], 6, None,
                                    op0=Alu.logical_shift_left)
            nc.vector.tensor_tensor(idx32[:], idx32[:], l32[:], op=Alu.add)

            # relu guard (if a row has <32 positive preacts, the reference's
            # extra top-k entries are relu zeros; zero coefficients match it)
            nc.vector.tensor_scalar_max(vals[:], vals[:], 0.0)

            # diag(vals[:, c]) tiles, fp16
            diag = sb.tile([128, K, 128], dt.float16, tag="diag")
            nc.gpsimd.affine_select(
                out=diag[:],
                in_=vals[:, :, None].to_broadcast([128, K, 128]),
                pattern=[[0, K], [1, 128]],
                compare_op=Alu.is_equal, fill=0.0,
                base=0, channel_multiplier=-1)

            # compact decode: xhat_tile = sum_c diag(vals[:,c]) @ WdecT[idx[:,c]]
            pso = psB.tile([128, D], dt.float32, tag="decps")
            for c in range(K):
                g = sb.tile([128, D], dt.float16, tag="g")
                nc.gpsimd.indirect_dma_start(
                    out=g[:], out_offset=None,
                    in_=wdecT_d.ap(),
                    in_offset=bass.IndirectOffsetOnAxis(ap=idx32[:, c:c + 1],
                                                        axis=0))
                nc.tensor.matmul(pso[:, :512], lhsT=diag[:, c, :],
                                 rhs=g[:, :512],
                                 start=(c == 0), stop=(c == K - 1))
                nc.tensor.matmul(pso[:, 512:D], lhsT=diag[:, c, :],
                                 rhs=g[:, 512:D],
                                 start=(c == 0), stop=(c == K - 1))
            osb = stp.tile([128, D], dt.float32, tag="osb")
            if has_bdec:
                nc.vector.tensor_add(osb[:], pso[:], bdec_bc[:])
            else:
                nc.scalar.copy(osb[:], pso[:])
            nc.sync.dma_start(out_d.ap()[bt * 128:(bt + 1) * 128, :], osb[:])

        if HALVES == 1:
            groups = [list(range(NBT))]
        else:
            h = NBT // HALVES
            groups = [list(range(i * h, (i + 1) * h)) for i in range(HALVES)]
        for bts in groups:
            encode_blocks(bts)
            for bt in bts:
                tail(bt)

    from contextlib import ExitStack
    with tile.TileContext(nc) as tc:
        with ExitStack() as ctx:
            pools = (
                ctx.enter_context(tc.tile_pool(name="sb", bufs=2)),
                ctx.enter_context(tc.tile_pool(name="sb1", bufs=1)),
                ctx.enter_context(tc.tile_pool(name="stage", bufs=4)),
                ctx.enter_context(tc.tile_pool(name="psA", bufs=4, space="PSUM")),
                ctx.enter_context(tc.tile_pool(name="psB", bufs=2, space="PSUM")),
            )
            if repeat == 1:
                body(tc, pools)
            else:
                with tc.For_i(0, repeat, 1):
                    body(tc, pools)

    _fix_sync_waits(nc)
    return nc


def _get_runner(has_benc, has_bdec, repeat=1):
    key = (has_benc, has_bdec, repeat, ENC_MODE, HALVES)
    if key in _cache:
        return _cache[key]
    import jax
    from jax.sharding import Mesh, PartitionSpec
    from jax.experimental.shard_map import shard_map
    import concourse.mybir as mybir
    from concourse import bass2jax
    from concourse.bass2jax import _bass_exec_p, install_neuronx_cc_hook

    nc = _build(has_benc, has_bdec, repeat)
    install_neuronx_cc_hook()

    partition_name = (nc.partition_id_tensor.name
                      if nc.partition_id_tensor else None)
    in_names, out_names, out_avals, zero_outs = [], [], [], []
    for alloc in nc.m.functions[0].allocations:
        if not isinstance(alloc, mybir.MemoryLocationSet):
            continue
        name = alloc.memorylocations[0].name
        if alloc.kind == "ExternalInput":
            if name != partition_name:
                in_names.append(name)
        elif alloc.kind == "ExternalOutput":
            shape = tuple(alloc.tensor_shape)
            dtype = mybir.dt.np(alloc.dtype)
            out_names.append(name)
            out_avals.append(jax.core.ShapedArray(shape, dtype))
            zero_outs.append(np.zeros(shape, dtype))
    n_params = len(in_names)
    all_in = in_names + out_names
    if partition_name is not None:
        all_in = all_in + [partition_name]

    def _bodyfn(*args):
        operands = list(args)
        if partition_name is not None:
            operands.append(bass2jax.partition_id_tensor())
        outs = _bass_exec_p.bind(
            *operands, out_avals=tuple(out_avals), in_names=tuple(all_in),
            out_names=tuple(out_names), lowering_input_output_aliases=(),
            sim_require_finite=True, sim_require_nnan=True, nc=nc)
        return tuple(outs)

    try:
        devices = jax.devices("axon")[:N_CORES]
    except Exception:
        devices = jax.devices()[:N_CORES]
    mesh = Mesh(np.asarray(devices), ("core",))
    n_outs = len(out_names)
    fn = jax.jit(
        shard_map(_bodyfn, mesh=mesh,
                  in_specs=(PartitionSpec("core"),) * (n_params + n_outs),
                  out_specs=(PartitionSpec("core"),) * n_outs,
                  check_rep=False),
        keep_unused=True)
    sharding = jax.sharding.NamedSharding(mesh, PartitionSpec("core"))
    r = {"fn": fn, "in_names": in_names, "out_names": out_names,
         "zero_outs": zero_outs, "nc": nc, "sharding": sharding}
    _cache[key] = r
    return r


def _prep_host(x, W_enc, b_enc, W_dec, b_dec):
    x_eff = x - b_dec[None, :]
    xT_full = np.ascontiguousarray(x_eff.T, dtype=np.float32)      # [D, B]
    wencT = np.ascontiguousarray(W_enc.T, dtype=np.float32)        # [D, F]
    wdecT16 = np.ascontiguousarray(W_dec.T, dtype=np.float16)      # [F, D]
    beff = (b_enc.astype(np.float64)
            - W_enc.astype(np.float64) @ b_dec.astype(np.float64))
    beff = beff.astype(np.float32)[None, :]                        # [1, F]
    bdec = b_dec.astype(np.float32)[None, :]                       # [1, D]
    return xT_full, wencT, wdecT16, beff, bdec


def kernel(x, W_enc, b_enc, W_dec, b_dec, _repeat=1, _timeit=False):
    x = np.asarray(x, np.float32)
    W_enc = np.asarray(W_enc, np.float32)
    b_enc = np.asarray(b_enc, np.float32)
    W_dec = np.asarray(W_dec, np.float32)
    b_dec = np.asarray(b_dec, np.float32)
    xT_full, wencT, wdecT16, beff, bdec = _prep_host(x, W_enc, b_enc, W_dec, b_dec)
    has_benc = bool(np.any(beff))
    has_bdec = bool(np.any(b_dec))
    r = _get_runner(has_benc, has_bdec, _repeat)

    per_core = {
        "wdecT16": [wdecT16] * N_CORES,
        "beff": [beff] * N_CORES,
        "bdec": [bdec] * N_CORES,
    }
    if ENC_MODE == "bf16x3":
        import ml_dtypes
        bf16 = ml_dtypes.bfloat16
        xTh = xT_full.astype(bf16)
        xTl = (xT_full - xTh.astype(np.float32)).astype(bf16)
        wh = wencT.astype(bf16)
        wl = (wencT - wh.astype(np.float32)).astype(bf16)
        per_core["xTh"] = [np.ascontiguousarray(xTh[:, c * BC:(c + 1) * BC])
                           for c in range(N_CORES)]
        per_core["xTl"] = [np.ascontiguousarray(xTl[:, c * BC:(c + 1) * BC])
                           for c in range(N_CORES)]
        per_core["wencTh"] = [wh] * N_CORES
        per_core["wencTl"] = [wl] * N_CORES
    else:
        per_core["xT"] = [np.ascontiguousarray(xT_full[:, c * BC:(c + 1) * BC])
                          for c in range(N_CORES)]
        per_core["wencT"] = [wencT] * N_CORES
    args = [np.concatenate(per_core[name], axis=0) for name in r["in_names"]]
    args += [np.concatenate([z] * N_CORES, axis=0) for z in r["zero_outs"]]

    import jax, time
    dev_args = [jax.device_put(a, r["sharding"]) for a in args]
    kernel.last_dev_args = dev_args
    kernel.last_runner = r
    outs = r["fn"](*dev_args)
    jax.block_until_ready(outs)
    if _timeit:
        times = []
        for _ in range(_timeit if isinstance(_timeit, int) and _timeit > 1 else 8):
            t0 = time.perf_counter()
            outs = r["fn"](*dev_args)
            jax.block_until_ready(outs)
            times.append(time.perf_counter() - t0)
        kernel.last_times = times

    xhat = np.asarray(outs[r["out_names"].index("xhat")])  # [B, D] concat
    return xhat.astype(np.float32)
